# revision 6
# baseline (speedup 1.0000x reference)
"""ClothLinearFusion Trainium2 kernel.

Computes out[b, i] = (sum_k cloth[b, k, i]) * (sum_j f[i, j] * body[b, j])
for cloth (128, 64, 1024), body (128, 1024), f (1024, 1024), all fp32.

Sharding: split the cloth-channel dim C=1024 into 8 slices of 128, one per
NeuronCore. Each core reads its cloth slice (4 MB), its slice of f.T
(0.5 MB) and the full body.T (0.5 MB) — 5 MB/core, vs 8 MB/core for
batch-parallel sharding (which would replicate all of f).

Host-side prep (numpy): slice cloth per core; transpose f and body so the
contraction dim j lands on SBUF partitions (PE contracts over partitions,
and fp32 has no DMA-transpose path on chip).
"""

import sys

sys.path.insert(0, "/opt/trn_rl_repo")

import numpy as np

import bass_rust
import concourse.bass as bass
import concourse.mybir as mybir
import concourse.tile as tile
from concourse.bass_utils import run_bass_kernel_spmd
from concourse.vector_clock import ScopedClock


def _split_drain_and_barrier(self, tick_clock, wait_clock):
    """Replacement for TileContext._drain_and_barrier.

    The stock version attaches one sem wait per allocated semaphore to a
    single SP drain; the walrus in this container enforces ONE sync wait
    per instruction, so split into one drain per semaphore.
    """
    nc = self.nc
    drain_inst = nc.sync.drain()
    wait_clock.add_sem_waits(
        drain_inst.ins, ScopedClock({None: tick_clock.global_clock})
    )
    si = drain_inst.ins.sync_info
    if si is not None and len(si.on_wait) > 1:
        waits = list(si.on_wait)
        drain_inst.ins.sync_info = bass_rust.SyncInfo(
            on_wait=waits[:1], on_update=list(si.on_update)
        )
        for w in waits[1:]:
            extra = nc.sync.drain()
            extra.ins.sync_info = bass_rust.SyncInfo(on_wait=[w], on_update=[])

    nc.all_engine_barrier()
    assert self.sems is not None
    popped = nc._tile_sem_poison_stack.pop()
    assert popped is self._sem_poison
    nc.clear_and_free_semaphores(list(self.sems.allocated().values()))
    nc.all_engine_barrier()


tile.TileContext._drain_and_barrier = _split_drain_and_barrier


def _assert_single_waits(nc):
    """Walrus rejects >1 sync wait on any instruction — fail fast at build."""
    for fn in nc.m.functions:
        for blk in fn.blocks:
            for inst in blk.instructions:
                si = inst.sync_info
                if si is not None and len(si.on_wait) > 1:
                    raise AssertionError(
                        f"{type(inst).__name__} {inst.name} has "
                        f"{len(si.on_wait)} waits: "
                        f"{[(w.ant_name, w.wait_value) for w in si.on_wait]}"
                    )

B = 128          # batch
K = 64           # cloth latent count (summed away)
C = 1024         # cloth channels
J = 1024         # body channels
NCORES = 8
CI = C // NCORES  # cloth channels per core = 128
KC = 4           # k-chunks for DMA/compute overlap

F32 = mybir.dt.float32

_CACHE = {}


def _build_program():
    """Bass program, identical across the 8 cores (SPMD)."""
    nc = bass.Bass(target_bir_lowering=False, debug=False)

    cloth = nc.dram_tensor("cloth_s", [B, K, CI], F32, kind="ExternalInput")
    # bf = concat([body.T, f_slice.T], axis=1): [j, b | ci] — one DMA so the
    # first matmul needs a single semaphore wait (LDW supports only one).
    bf = nc.dram_tensor("bf_s", [J, B + CI], F32, kind="ExternalInput")
    out = nc.dram_tensor("out_s", [B, CI], F32, kind="ExternalOutput")

    JCH = J // 128  # 8 chunks of the contraction dim
    KS = K // KC    # k-values per chunk

    with tile.TileContext(nc) as tc:
        with (
            tc.tile_pool(name="pool", bufs=1) as pool,
            tc.tile_pool(name="psum", bufs=1, space=bass.MemorySpace.PSUM) as psum_pool,
        ):
            # ---- fv[b, ci] = sum_j body[b, j] * f[ci, j] on PE ----
            bft = pool.tile([128, JCH, B + CI], F32)  # [j%128, j//128, b|ci]
            nc.sync.dma_start(
                out=bft[:], in_=bf.rearrange("(c p) n -> p c n", p=128)
            )

            fv_psum = psum_pool.tile([B, CI], F32)
            for c in range(JCH):
                nc.tensor.matmul(
                    fv_psum[:],
                    bft[:, c, 0:B],
                    bft[:, c, B:B + CI],
                    start=(c == 0),
                    stop=(c == JCH - 1),
                )

            # ---- c_sum[b, ci] = sum_k cloth[b, k, ci] on DVE, k-chunked ----
            partial = pool.tile([B, KC, CI], F32)
            for q in range(KC):
                chunk = pool.tile([B, KS, CI], F32, tag=f"clchunk{q}")
                nc.sync.dma_start(out=chunk[:], in_=cloth[:, q * KS:(q + 1) * KS, :])
                nc.vector.reduce_sum(
                    out=partial[:, q, :],
                    in_=chunk.rearrange("p k n -> p n k"),
                    axis=mybir.AxisListType.X,
                )
            csum = pool.tile([B, CI], F32)
            nc.vector.reduce_sum(
                out=csum[:],
                in_=partial.rearrange("p q n -> p n q"),
                axis=mybir.AxisListType.X,
            )

            # ---- out = c_sum * fv ----
            # TRN2 instructions carry at most ONE semaphore wait. Copy fv
            # PSUM->SBUF on DVE first (single PE wait); then the mul's
            # operands are both DVE-produced (single DVE self-wait).
            fv_sb = pool.tile([B, CI], F32)
            nc.vector.tensor_copy(out=fv_sb[:], in_=fv_psum[:])
            res = pool.tile([B, CI], F32)
            nc.vector.tensor_mul(out=res[:], in0=csum[:], in1=fv_sb[:])
            nc.sync.dma_start(out=out[:], in_=res[:])

    _assert_single_waits(nc)
    return nc


def _get_program():
    if "nc" not in _CACHE:
        _CACHE["nc"] = _build_program()
    return _CACHE["nc"]


def _make_in_maps(cloth_latent, body_latent, f):
    cloth_latent = np.ascontiguousarray(np.asarray(cloth_latent, dtype=np.float32))
    body_latent = np.ascontiguousarray(np.asarray(body_latent, dtype=np.float32))
    f = np.asarray(f, dtype=np.float32)

    bodyT = body_latent.T                                # (J, B) view
    fT = f.T                                             # (J, C) view

    in_maps = []
    for i in range(NCORES):
        sl = slice(i * CI, (i + 1) * CI)
        bf = np.concatenate([bodyT, fT[:, sl]], axis=1)  # (J, B + CI) contiguous
        in_maps.append(
            {
                "cloth_s": np.ascontiguousarray(cloth_latent[:, :, sl]),
                "bf_s": bf,
            }
        )
    return in_maps


def _run(cloth_latent, body_latent, f, trace=False):
    nc = _get_program()
    in_maps = _make_in_maps(cloth_latent, body_latent, f)
    r = run_bass_kernel_spmd(nc, in_maps, list(range(NCORES)), trace=trace)
    out = np.concatenate([r.results[i]["out_s"] for i in range(NCORES)], axis=1)
    return out, r


def kernel(cloth_latent, body_latent, f):
    out, _ = _run(cloth_latent, body_latent, f, trace=False)
    return out


def kernel_traced(cloth_latent, body_latent, f):
    """Returns (output, BassKernelResults) with NTFF profiling enabled."""
    return _run(cloth_latent, body_latent, f, trace=True)


# revision 7
# speedup vs baseline: 1.4477x; 1.4477x over previous
"""ClothLinearFusion Trainium2 kernel.

Computes out[b, i] = (sum_k cloth[b, k, i]) * (sum_j f[i, j] * body[b, j])
for cloth (128, 64, 1024), body (128, 1024), f (1024, 1024), all fp32.

Sharding: split the cloth-channel dim C=1024 into 8 slices of 128, one per
NeuronCore. Each core reads its cloth slice (4 MB), its slice of f.T
(0.5 MB) and the full body.T (0.5 MB) — 5 MB/core, vs 8 MB/core for
batch-parallel sharding (which would replicate all of f).

Host-side prep (numpy): slice cloth per core (contiguous per-partition
slabs); build bf = concat([body.T, fT_slice], axis=1) pre-swizzled to
[p, jchunk, 256] so its DMA uses 8 KB-contiguous descriptors and the
PE contraction dim j lands on SBUF partitions.

All HWDGE DMAs ride the single qSPDynamicHW ring in FIFO order: bf first
(unblocks the 8 fp32 matmuls), then cloth k-chunks sized so the DVE
tree-reduction pipelines behind the DMA stream with a short tail.
"""

import sys

sys.path.insert(0, "/opt/trn_rl_repo")

import numpy as np

import bass_rust
import concourse.bass as bass
import concourse.mybir as mybir
import concourse.tile as tile
from concourse.bass_utils import run_bass_kernel_spmd
from concourse.vector_clock import ScopedClock

B = 128          # batch
K = 64           # cloth latent count (summed away)
C = 1024         # cloth channels
J = 1024         # body channels
NCORES = 8
CI = C // NCORES  # cloth channels per core = 128
KCHUNKS = [16, 16, 16, 8, 4, 4]  # k-chunk sizes: big while streaming, small tail

F32 = mybir.dt.float32

_CACHE = {}


# ---------------------------------------------------------------------------
# Framework patches for this container's walrus (ONE sync wait per
# instruction) and slow GpSimd teardown.
# ---------------------------------------------------------------------------

def _split_drain_and_barrier(self, tick_clock, wait_clock):
    """TileContext._drain_and_barrier with the multi-sem wait split into one
    drain per semaphore (walrus here rejects >1 sync wait per instruction)."""
    nc = self.nc
    drain_inst = nc.sync.drain()
    wait_clock.add_sem_waits(
        drain_inst.ins, ScopedClock({None: tick_clock.global_clock})
    )
    si = drain_inst.ins.sync_info
    if si is not None and len(si.on_wait) > 1:
        waits = list(si.on_wait)
        drain_inst.ins.sync_info = bass_rust.SyncInfo(
            on_wait=waits[:1], on_update=list(si.on_update)
        )
        for w in waits[1:]:
            extra = nc.sync.drain()
            extra.ins.sync_info = bass_rust.SyncInfo(on_wait=[w], on_update=[])

    nc.all_engine_barrier()
    assert self.sems is not None
    popped = nc._tile_sem_poison_stack.pop()
    assert popped is self._sem_poison
    nc.clear_and_free_semaphores(list(self.sems.allocated().values()))
    nc.all_engine_barrier()


tile.TileContext._drain_and_barrier = _split_drain_and_barrier


def _compact_to_ranges(nums):
    nums = sorted(set(nums))
    ranges = []
    start = prev = nums[0]
    for n in nums[1:]:
        if n == prev + 1:
            prev = n
            continue
        ranges.append(range(start, prev + 1))
        start = prev = n
    ranges.append(range(start, prev + 1))
    return ranges


def _fast_clear_and_free_semaphores(self, sems):
    """Bass.clear_and_free_semaphores via SP instead of GpSimd — the Q7
    dma_reset + sem_clear pair costs ~3.5 us each on Pool."""
    if not sems:
        return
    sem_nums = [s.num if hasattr(s, "num") else s for s in sems]
    for sem_range in _compact_to_ranges(sem_nums):
        assert self._state.free_isdisjoint(sem_range)
        self.sync.drain(semaphore_range=sem_range)
        self.sync.sem_clear(sem_range)
    self._state.prepend_free_semaphores(sem_nums)
    for poison_set in self._tile_sem_poison_stack:
        poison_set.update(sem_nums)


def _strip_preamble(nc):
    """Remove the const-AP memsets (unused here; ~3.5 us of GpSimd time) and
    the initial all-engine barrier from the Bass preamble. Cross-engine
    ordering inside the kernel body is fully sem-managed by Tile."""
    main_blk = None
    for fn in nc.m.functions:
        for blk in fn.blocks:
            if blk.name == "main":
                main_blk = blk
    assert main_blk is not None
    to_drop = []
    for inst in main_blk.instructions:
        t = type(inst).__name__
        if t == "InstMemset":
            to_drop.append(inst)
        elif t in ("InstDrain", "InstEventSemaphore"):
            to_drop.append(inst)
    for inst in to_drop:
        main_blk.instructions.remove(inst)


def _assert_single_waits(nc):
    for fn in nc.m.functions:
        for blk in fn.blocks:
            for inst in blk.instructions:
                si = inst.sync_info
                if si is not None and len(si.on_wait) > 1:
                    raise AssertionError(
                        f"{type(inst).__name__} {inst.name} has "
                        f"{len(si.on_wait)} waits: "
                        f"{[(w.ant_name, w.wait_value) for w in si.on_wait]}"
                    )


# ---------------------------------------------------------------------------
# Kernel program (SPMD, identical on all 8 cores)
# ---------------------------------------------------------------------------

def _build_program():
    nc = bass.Bass(target_bir_lowering=False, debug=False)
    nc.clear_and_free_semaphores = _fast_clear_and_free_semaphores.__get__(nc)

    cloth = nc.dram_tensor("cloth_s", [B, K, CI], F32, kind="ExternalInput")
    # bf_s[p, c, 0:128] = body.T[c*128+p, :], bf_s[p, c, 128:256] = fT_slice
    bf = nc.dram_tensor("bf_s", [128, J // 128, B + CI], F32, kind="ExternalInput")
    out = nc.dram_tensor("out_s", [B, CI], F32, kind="ExternalOutput")

    JCH = J // 128

    with tile.TileContext(nc) as tc:
        with (
            tc.tile_pool(name="pool", bufs=1) as pool,
            tc.tile_pool(name="tree", bufs=2) as tree_pool,
            tc.tile_pool(name="psum", bufs=1, space=bass.MemorySpace.PSUM) as psum_pool,
        ):
            # --- DMA issue order == qSPDynamicHW FIFO order ---
            bft = pool.tile([128, JCH, B + CI], F32)
            nc.sync.dma_start(out=bft[:], in_=bf[:])

            chunks = []
            k0 = 0
            for q, ks in enumerate(KCHUNKS):
                ch = pool.tile([B, ks, CI], F32, tag=f"ch{q}")
                nc.sync.dma_start(out=ch[:], in_=cloth[:, k0:k0 + ks, :])
                chunks.append((ch, ks))
                k0 += ks

            # --- fv[b, ci] = sum_j body[b, j] * f[ci, j] on PE ---
            fv_psum = psum_pool.tile([B, CI], F32)
            for c in range(JCH):
                nc.tensor.matmul(
                    fv_psum[:],
                    bft[:, c, 0:B],
                    bft[:, c, B:B + CI],
                    start=(c == 0),
                    stop=(c == JCH - 1),
                )

            # --- c_sum via DVE binary-tree adds, chunk-pipelined ---
            acc = pool.tile([B, CI], F32)
            fv_sb = pool.tile([B, CI], F32)
            for q, (ch, ks) in enumerate(chunks):
                cur = ch
                n = ks
                while n > 2:
                    half = n // 2
                    t = tree_pool.tile([B, half, CI], F32, tag=f"t{half}")
                    nc.vector.tensor_add(
                        out=t[:], in0=cur[:, 0:half, :], in1=cur[:, half:n, :]
                    )
                    cur, n = t, half
                # last level writes the chunk partial
                partial = tree_pool.tile([B, CI], F32, tag="partial")
                nc.vector.tensor_add(
                    out=partial[:], in0=cur[:, 0, :], in1=cur[:, 1, :]
                )
                if q == 0:
                    first_partial = partial
                elif q == 1:
                    nc.vector.tensor_add(
                        out=acc[:], in0=first_partial[:], in1=partial[:]
                    )
                else:
                    nc.vector.tensor_add(out=acc[:], in0=acc[:], in1=partial[:])
                if q == 2:
                    # DVE idle gap while chunk 3 streams: PSUM -> SBUF copy of
                    # fv (single PE wait here; keeps the final mul single-wait)
                    nc.vector.tensor_copy(out=fv_sb[:], in_=fv_psum[:])

            # --- out = c_sum * fv ---
            res = pool.tile([B, CI], F32)
            nc.vector.tensor_mul(out=res[:], in0=acc[:], in1=fv_sb[:])
            nc.sync.dma_start(out=out[:], in_=res[:])

    _strip_preamble(nc)
    _assert_single_waits(nc)
    return nc


def _get_program():
    if "nc" not in _CACHE:
        _CACHE["nc"] = _build_program()
    return _CACHE["nc"]


def _make_in_maps(cloth_latent, body_latent, f):
    cloth_latent = np.ascontiguousarray(np.asarray(cloth_latent, dtype=np.float32))
    body_latent = np.asarray(body_latent, dtype=np.float32)
    f = np.asarray(f, dtype=np.float32)

    bodyT = body_latent.T                                # (J, B) view
    fT = f.T                                             # (J, C) view

    in_maps = []
    for i in range(NCORES):
        sl = slice(i * CI, (i + 1) * CI)
        bf = np.concatenate([bodyT, fT[:, sl]], axis=1)  # (J, B + CI)
        # swizzle to [p, jchunk, B+CI]: row j = c*128 + p
        bf_r = np.ascontiguousarray(
            bf.reshape(J // 128, 128, B + CI).transpose(1, 0, 2)
        )
        in_maps.append(
            {
                "cloth_s": np.ascontiguousarray(cloth_latent[:, :, sl]),
                "bf_s": bf_r,
            }
        )
    return in_maps


def _run(cloth_latent, body_latent, f, trace=False):
    nc = _get_program()
    in_maps = _make_in_maps(cloth_latent, body_latent, f)
    r = run_bass_kernel_spmd(nc, in_maps, list(range(NCORES)), trace=trace)
    out = np.concatenate([r.results[i]["out_s"] for i in range(NCORES)], axis=1)
    return out, r


def kernel(cloth_latent, body_latent, f):
    out, _ = _run(cloth_latent, body_latent, f, trace=False)
    return out


def kernel_traced(cloth_latent, body_latent, f):
    """Returns (output, BassKernelResults) with NTFF profiling enabled."""
    return _run(cloth_latent, body_latent, f, trace=True)
